# revision 1
# baseline (speedup 1.0000x reference)
"""Trainium2 Bass kernel for nn_Attention (tanh-attention pooling + MLP classifier).

Computation (fp32 reference):
    u      = tanh(emb @ W + bias)            [B,T,H]
    scores = tanh(u @ c)[..., 0]             [B,T]
    attn   = softmax(scores, axis=1)
    ctx    = einsum('bth,bt->bh', emb, attn) [B,H]
    out    = softmax(relu(ctx@W1 + b1) @ W2 + b2, axis=1)

Strategy: data-parallel over batch (8 cores x 8 batches each). The embedding
is uploaded pre-transposed/interleaved [b, p, hc, t] in bf16 so the big
matmul streams it directly (contraction over H on partitions, h = hc*128+p)
and the attention-weighted pooling reduces over T along the free axis with
the fused affine_mul_reduce DVE op. z^T PSUM tiles are grouped per j-chunk
spanning two t-chunks so tanh's per-partition bias argument applies the
model bias for free. Scores are accumulated into PSUM partitions 32*tc
(tile_position column packing) so the per-batch tanh/exp run as single
128-lane ACT ops; exp's accum_out row-sums feed the softmax normalizer,
reduced across the four packed rows with a masked-ones matmul. attn rows
are replicated across partitions with a ones-column outer product on the
PE; softmax normalization is deferred to the classifier head.
"""

import sys

if "/opt/trn_rl_repo" not in sys.path:
    sys.path.insert(0, "/opt/trn_rl_repo")

import numpy as np

B, T, HID = 64, 2048, 512
H_CLS, D_OUT = 1024, 10
N_CORES = 8
BPC = B // N_CORES          # batches per core
NTC = T // 512              # t-chunks per batch
NHC = HID // 128            # h chunks
NJC = HID // 128            # j chunks

_CACHED = {}


def _build_nc():
    import contextlib

    import concourse.bacc as bacc
    import concourse.tile as tile
    from concourse import mybir

    f32 = mybir.dt.float32
    bf16 = mybir.dt.bfloat16
    AF = mybir.ActivationFunctionType
    Alu = mybir.AluOpType
    X = mybir.AxisListType.X

    nc = bacc.Bacc(None)

    embT = nc.dram_tensor("embT", [BPC, 128, NHC, T], bf16, kind="ExternalInput")
    w_d = nc.dram_tensor("w", [HID, HID], bf16, kind="ExternalInput")
    bias_d = nc.dram_tensor("bias", [128, NJC], f32, kind="ExternalInput")
    c_d = nc.dram_tensor("c", [128, NJC], bf16, kind="ExternalInput")
    w1_d = nc.dram_tensor("w1", [HID, H_CLS], bf16, kind="ExternalInput")
    b1_d = nc.dram_tensor("b1", [1, H_CLS], bf16, kind="ExternalInput")
    w2_d = nc.dram_tensor("w2", [128, H_CLS // 128, D_OUT], bf16, kind="ExternalInput")
    b2_d = nc.dram_tensor("b2", [1, D_OUT], bf16, kind="ExternalInput")
    sel_d = nc.dram_tensor("sel", [128, 1], f32, kind="ExternalInput")
    out_d = nc.dram_tensor("out", [BPC, D_OUT], f32, kind="ExternalOutput")

    with tile.TileContext(nc) as tc:
        with contextlib.ExitStack() as ctx:
            wpool = ctx.enter_context(tc.tile_pool(name="wpool", bufs=1))
            etp = ctx.enter_context(tc.tile_pool(name="etp", bufs=3))
            utp = ctx.enter_context(tc.tile_pool(name="utp", bufs=3))
            sbp = ctx.enter_context(tc.tile_pool(name="sbp", bufs=2))
            repp = ctx.enter_context(tc.tile_pool(name="repp", bufs=2))
            ctxp = ctx.enter_context(tc.tile_pool(name="ctxp", bufs=1))
            ps = ctx.enter_context(tc.tile_pool(name="ps", bufs=1, space="PSUM"))

            # ---- weights / constants to SBUF ----
            w_sb = []
            w1_sb = []
            for hc in range(NHC):
                wt = wpool.tile([128, HID], bf16, name=f"w_sb{hc}")
                nc.sync.dma_start(out=wt[:], in_=w_d[128 * hc : 128 * (hc + 1), :])
                w_sb.append(wt)
                w1t = wpool.tile([128, H_CLS], bf16, name=f"w1_sb{hc}")
                nc.sync.dma_start(out=w1t[:], in_=w1_d[128 * hc : 128 * (hc + 1), :])
                w1_sb.append(w1t)
            bias_sb = wpool.tile([128, NJC], f32, name="bias_sb")
            nc.sync.dma_start(out=bias_sb[:], in_=bias_d[:])
            c_sb = wpool.tile([128, NJC], bf16, name="c_sb")
            nc.sync.dma_start(out=c_sb[:], in_=c_d[:])
            w2_sb = wpool.tile([128, H_CLS // 128, D_OUT], bf16, name="w2_sb")
            nc.sync.dma_start(out=w2_sb[:], in_=w2_d[:])
            b1_sb = wpool.tile([1, H_CLS], bf16, name="b1_sb")
            nc.sync.dma_start(out=b1_sb[:], in_=b1_d[:])
            b2_sb = wpool.tile([1, D_OUT], bf16, name="b2_sb")
            nc.sync.dma_start(out=b2_sb[:], in_=b2_d[:])
            sel_sb = wpool.tile([128, 1], f32, name="sel_sb")
            nc.sync.dma_start(out=sel_sb[:], in_=sel_d[:])
            ones_sb = wpool.tile([128, 128], bf16, name="ones_sb")
            nc.vector.memset(ones_sb[:], 1.0)
            onesf_sb = wpool.tile([1, 128], f32, name="onesf_sb")
            nc.vector.memset(onesf_sb[:], 1.0)

            # persistent accumulators
            ctxt = ctxp.tile([128, NHC, BPC], f32, name="ctxt")
            psum_S = ps.tile([1, BPC], f32, name="psum_S", tag="S")

            # ---- main loop ----
            pz_cm = tc.tile_pool(name="pz", bufs=2, space="PSUM")
            pz = pz_cm.__enter__()
            for b in range(BPC):
                psc = ps.tile([128, 512], f32, name=f"psc{b}", tag="scores", bufs=2)
                nc.vector.memset(psc[:], 0.0)
                et = etp.tile([128, NHC, T], bf16, name=f"et{b}", tag="et")
                for ks in range(NHC):
                    nc.sync.dma_start(out=et[:, ks, :], in_=embT[b, :, ks, :])

                for tp in range(NTC // 2):
                    # z^T per j-chunk over both halves; tanh applies the bias
                    for jc in range(NJC):
                        pzt = pz.tile([128, 1024], f32, name=f"pz{b}_{tp}_{jc}", tag="z")
                        for tch in range(2):
                            tcN = 2 * tp + tch
                            for hc in range(NHC):
                                nc.tensor.matmul(
                                    pzt[:, 512 * tch : 512 * (tch + 1)],
                                    w_sb[hc][:, 128 * jc : 128 * (jc + 1)],
                                    et[:, hc, 512 * tcN : 512 * (tcN + 1)],
                                    start=(hc == 0),
                                    stop=(hc == NHC - 1),
                                )
                        ut = utp.tile([128, 1024], bf16, name=f"ut{b}_{tp}_{jc}", tag="ut")
                        nc.scalar.activation(
                            ut[:], pzt[:], AF.Tanh, bias=bias_sb[:, jc : jc + 1]
                        )
                        # scores: accumulate c . u^T into row 32*tcN
                        for tch in range(2):
                            tcN = 2 * tp + tch
                            nc.tensor.matmul(
                                psc[32 * tcN : 32 * tcN + 1, :],
                                c_sb[:, jc : jc + 1],
                                ut[:, 512 * tch : 512 * (tch + 1)],
                                start=(jc == 0),
                                stop=(jc == NJC - 1),
                                tile_position=(0, 32 * tcN),
                            )

                # batch-level: attn weights + row sums (valid rows: 0/32/64/96;
                # the rest were memset to 0 -> exp(tanh(0))=1, masked by sel)
                tanh_s = sbp.tile([128, 512], f32, name=f"tanh_s{b}", tag="tanh_s")
                nc.scalar.activation(tanh_s[:], psc[:], AF.Tanh)
                attn = sbp.tile([128, 512], bf16, name=f"attn{b}", tag="attn")
                sS = sbp.tile([128, 1], f32, name=f"sS{b}", tag="sS")
                nc.scalar.activation(attn[:], tanh_s[:], AF.Exp, accum_out=sS[:])
                # S_b = sum of rows {0,32,64,96} via masked ones matmul
                nc.tensor.matmul(
                    psum_S[0:1, b : b + 1], sS[:], sel_sb[:], start=True, stop=True
                )

                # pooling: replicate attn rows then fused mul-reduce over T
                rep = repp.tile([128, NTC, 512], bf16, name=f"rep{b}", tag="rep")
                for tcN in range(NTC):
                    prep = ps.tile([128, 512], f32, name=f"prep{b}_{tcN}", tag="rep")
                    nc.tensor.matmul(
                        prep[:],
                        ones_sb[32 * tcN : 32 * tcN + 1, :],
                        attn[32 * tcN : 32 * tcN + 1, :],
                        start=True,
                        stop=True,
                        tile_position=(32 * tcN, 0),
                    )
                    nc.vector.tensor_copy(rep[:, tcN, :], prep[:])
                rep_flat = rep.rearrange("p a b -> p (a b)")
                for hc in range(NHC):
                    amro = repp.tile(
                        [128, T], bf16, name=f"amro{b}_{hc}", tag="amro", bufs=2
                    )
                    nc.vector.affine_mul_reduce(
                        out=amro[:],
                        accum_out=ctxt[:, hc, b : b + 1],
                        in0=et[:, hc, :],
                        in1=rep_flat,
                        scale=1.0,
                        bias=0.0,
                    )

            pz_cm.__exit__(None, None, None)
            # ---- tail: normalize + classifier + softmax ----
            recipS = ctxp.tile([1, BPC], f32, name="recipS")
            nc.vector.reciprocal(recipS[:], psum_S[:])
            with tc.tile_pool(name="pt", bufs=1, space="PSUM") as pt:
                rsrep = pt.tile([128, BPC], f32, name="rsrep", tag="rsrep")
                nc.tensor.matmul(rsrep[:], onesf_sb[:], recipS[:], start=True, stop=True)
                ctxn = ctxp.tile([128, NHC, BPC], bf16, name="ctxn")
                for hc in range(NHC):
                    nc.vector.tensor_mul(ctxn[:, hc, :], ctxt[:, hc, :], rsrep[:])

                # h1^T [1024, 8] in chunks of 128 rows: psum [128, 64]
                ph1 = pt.tile([128, (H_CLS // 128) * BPC], f32, name="ph1", tag="h1")
                for ncc in range(H_CLS // 128):
                    for hc in range(NHC):
                        nc.tensor.matmul(
                            ph1[:, BPC * ncc : BPC * (ncc + 1)],
                            w1_sb[hc][:, 128 * ncc : 128 * (ncc + 1)],
                            ctxn[:, hc, :],
                            start=(hc == 0),
                            stop=False,
                        )
                    nc.tensor.matmul(
                        ph1[:, BPC * ncc : BPC * (ncc + 1)],
                        b1_sb[0:1, 128 * ncc : 128 * (ncc + 1)],
                        ones_sb[0:1, 0:BPC],
                        start=False,
                        stop=True,
                    )
                h1t = ctxp.tile([128, (H_CLS // 128) * BPC], bf16, name="h1t")
                nc.scalar.activation(h1t[:], ph1[:], AF.Relu)

                plog = pt.tile([BPC, D_OUT], f32, name="plog", tag="log")
                for ncc in range(H_CLS // 128):
                    nc.tensor.matmul(
                        plog[:],
                        h1t[:, BPC * ncc : BPC * (ncc + 1)],
                        w2_sb[:, ncc, :],
                        start=(ncc == 0),
                        stop=False,
                    )
                nc.tensor.matmul(
                    plog[:], ones_sb[0:1, 0:BPC], b2_sb[0:1, :], start=False, stop=True
                )

                # softmax over D_OUT (free axis)
                mx = ctxp.tile([BPC, 1], f32, name="mx")
                nc.vector.tensor_reduce(out=mx[:], in_=plog[:], axis=X, op=Alu.max)
                negmx = ctxp.tile([BPC, 1], f32, name="negmx")
                nc.vector.tensor_scalar_mul(negmx[:], mx[:], -1.0)
                esb = ctxp.tile([BPC, D_OUT], f32, name="esb")
                ssum = ctxp.tile([BPC, 1], f32, name="ssum")
                nc.scalar.activation(
                    esb[:], plog[:], AF.Exp, bias=negmx[:], scale=1.0, accum_out=ssum[:]
                )
                rsum = ctxp.tile([BPC, 1], f32, name="rsum")
                nc.vector.reciprocal(rsum[:], ssum[:])
                osb = ctxp.tile([BPC, D_OUT], f32, name="osb")
                nc.vector.tensor_scalar_mul(osb[:], esb[:], rsum[:])
                nc.sync.dma_start(out=out_d[:], in_=osb[:])

    nc.finalize()
    return nc


def _get_nc():
    if "nc" not in _CACHED:
        _CACHED["nc"] = _build_nc()
    return _CACHED["nc"]


def _prep_in_maps(embedding, weight, bias, context_weight, W1, b1, W2, b2):
    import ml_dtypes

    bf16 = ml_dtypes.bfloat16

    emb = np.asarray(embedding, dtype=np.float32)
    Wf = np.asarray(weight, dtype=np.float32)
    bf = np.asarray(bias, dtype=np.float32).reshape(HID)
    cf = np.asarray(context_weight, dtype=np.float32).reshape(HID)
    W1f = np.asarray(W1, dtype=np.float32)
    b1f = np.asarray(b1, dtype=np.float32).reshape(H_CLS)
    W2f = np.asarray(W2, dtype=np.float32)
    b2f = np.asarray(b2, dtype=np.float32).reshape(D_OUT)

    w_np = np.ascontiguousarray(Wf).astype(bf16)             # [512,512]
    bias_np = np.ascontiguousarray(bf.reshape(NJC, 128).T)   # [128,4] f32
    c_np = np.ascontiguousarray(cf.reshape(NJC, 128).T).astype(bf16)
    w1_np = np.ascontiguousarray(W1f).astype(bf16)           # [512,1024]
    b1_np = b1f.reshape(1, H_CLS).astype(bf16)
    w2_np = np.ascontiguousarray(
        W2f.reshape(H_CLS // 128, 128, D_OUT).transpose(1, 0, 2)
    ).astype(bf16)                                           # [128,8,10]
    b2_np = b2f.reshape(1, D_OUT).astype(bf16)
    sel_np = np.zeros((128, 1), np.float32)
    sel_np[[32 * i for i in range(NTC)], 0] = 1.0

    in_maps = []
    for i in range(N_CORES):
        shard = emb[BPC * i : BPC * (i + 1)]                  # [8,2048,512]
        # [b, t, h] -> [b, h, t] -> [b, hc, p, t] -> [b, p, hc, t]
        embT_np = np.ascontiguousarray(
            shard.transpose(0, 2, 1)
            .reshape(BPC, NHC, 128, T)
            .transpose(0, 2, 1, 3)
        ).astype(bf16)
        in_maps.append(
            {
                "embT": embT_np,
                "w": w_np,
                "bias": bias_np,
                "c": c_np,
                "w1": w1_np,
                "b1": b1_np,
                "w2": w2_np,
                "b2": b2_np,
                "sel": sel_np,
            }
        )
    return in_maps


def kernel(numerical, embedding, weight, bias, context_weight, W1, b1, W2, b2):
    from concourse.bass_utils import run_bass_kernel_spmd

    in_maps = _prep_in_maps(embedding, weight, bias, context_weight, W1, b1, W2, b2)
    nc = _get_nc()
    res = run_bass_kernel_spmd(nc, in_maps, list(range(N_CORES)))
    out = np.concatenate([res.results[i]["out"] for i in range(N_CORES)], axis=0)
    return out.astype(np.float32)

